# revision 12
# baseline (speedup 1.0000x reference)
"""CMAlign Trainium2 kernel (Bass/Tile, 8-core SPMD).

Sharding: one identity octet per core (samples {4b..4b+3, 32+4b..32+4b+3}).
pos pairs are octet-internal; neg rows gathered on host.

Device (per sample j):
  simT[s,t] = sum_c fn[j][c,s] * fn[pair][c,t]   (fp32r matmuls, rhs packs pos|neg, N=324)
  expT = exp(50*simT)                            (ScalarE)
  S[t] = sum_s expT  (ones-matmul, e_j-packed)   Z[t] = sum_s mask[s]*expT_pos
  A = mask/S (DVE, batched rows);  mp' = A*expT + (1-mask[t])*delta[s,t]
  recon[c,t] = sum_s fT_raw[s,c] * mp'[s,t]      -> PSUM [recon_pos | recon_neg]
  evac: pos fp32 (ScalarE), neg bf16 (VectorE)

Host: norms/masks/normalize + gathers (pre), distances/trip/comask/loss (post).
"""

import os
import numpy as np

B, P, C, H, W = 8, 4, 2048, 18, 9
N = 2 * B * P            # 64
HW = H * W               # 162
KC = C // 128            # 16 channel tiles
SQ = (128, 34)           # s-chunk sizes
TEMP = 50.0
MARGIN = 0.3
EPS_NORM = 1e-12
EPS_DIST = 1e-6
NCORES = 8

TRACE = False            # set True (e.g. from test.py) to profile
LAST = {}                # exec_time_ns etc. stashed here after a run


def _build_bass():
    import concourse.bacc as bacc
    import concourse.tile as tile
    import concourse.mybir as mybir

    dt = mybir.dt
    f32 = dt.float32
    f32r = dt.float32r
    bf16 = dt.bfloat16
    AF = mybir.ActivationFunctionType

    nc = bacc.Bacc("TRN2", target_bir_lowering=False, debug=False,
                   num_devices=NCORES)

    # ---- DRAM I/O (per-core shapes) ----
    fn_l = nc.dram_tensor("fn_l", [8, 128, KC * 162], f32r, kind="ExternalInput").ap()
    fnrhs = nc.dram_tensor("fnrhs", [8, 128, KC * 324], f32r, kind="ExternalInput").ap()
    ftq0 = nc.dram_tensor("ftq0", [8, 128, C], f32r, kind="ExternalInput").ap()
    ftq1 = nc.dram_tensor("ftq1", [8, 34, C], f32r, kind="ExternalInput").ap()
    dtq0 = nc.dram_tensor("dtq0", [8, 128, 324], f32, kind="ExternalInput").ap()
    dtq1 = nc.dram_tensor("dtq1", [8, 34, 324], f32, kind="ExternalInput").ap()
    eones = nc.dram_tensor("eones", [128, 128], f32, kind="ExternalInput").ap()
    emask = nc.dram_tensor("emask", [128, 128], f32, kind="ExternalInput").ap()
    erow = nc.dram_tensor("erow", [8, 1024], f32, kind="ExternalInput").ap()
    marow = nc.dram_tensor("marow", [8, 324], f32, kind="ExternalInput").ap()

    recpos = nc.dram_tensor("recpos", [8, 128, KC * 162], f32, kind="ExternalOutput").ap()
    recneg = nc.dram_tensor("recneg", [8, 128, KC * 162], bf16, kind="ExternalOutput").ap()
    srow_o = nc.dram_tensor("srow", [8, 324], f32, kind="ExternalOutput").ap()
    zrow_o = nc.dram_tensor("zrow", [8, 162], f32, kind="ExternalOutput").ap()

    def r(ap):
        return ap.bitcast(f32r)

    with tile.TileContext(nc) as tc:
        with (
            tc.tile_pool(name="const", bufs=1) as const,
            tc.tile_pool(name="sin", bufs=2) as sin,
            tc.tile_pool(name="sexp", bufs=1) as sexp,
            tc.tile_pool(name="sft", bufs=2) as sft,
            tc.tile_pool(name="smp", bufs=2) as smp,
            tc.tile_pool(name="sout", bufs=2) as sout,
            tc.tile_pool(name="srows", bufs=1) as srows,
            tc.tile_pool(name="psim", bufs=2, space="PSUM") as psim,
            tc.tile_pool(name="prec", bufs=2, space="PSUM") as prec,
            tc.tile_pool(name="prow", bufs=1, space="PSUM") as prow_pool,
            tc.tile_pool(name="pabc", bufs=1, space="PSUM") as pabc,
        ):
            # constants
            eones_t = const.tile([128, 128], f32)
            nc.sync.dma_start(eones_t[:], eones[:])
            emask_t = const.tile([128, 128], f32)
            nc.sync.dma_start(emask_t[:], emask[:])
            erow_t = const.tile([8, 1024], f32)
            nc.sync.dma_start(erow_t[:], erow[:])
            marow_t = const.tile([8, 324], f32)
            nc.sync.dma_start(marow_t[:], marow[:])

            prow = prow_pool.tile([8, 486], f32)  # cols 0:324 = S, 324:486 = Z

            exp_tiles = {}

            # ---------------- stage A: sim + exp + S/Z rows ----------------
            for j in range(8):
                fnl_t = sin.tile([128, KC * 162], f32r, tag="fnl")
                nc.sync.dma_start(fnl_t[:], fn_l[j])
                fnr_t = sin.tile([128, KC * 324], f32r, tag="fnr")
                nc.sync.dma_start(fnr_t[:], fnrhs[j])

                sims = [psim.tile([128, 324], f32, tag="sim0", name="sim0"),
                        psim.tile([128, 324], f32, tag="sim1", name="sim1")]
                for k in range(KC):
                    rhs = fnr_t[:, k * 324:(k + 1) * 324]
                    lhs0 = fnl_t[:, k * 162: k * 162 + 128]
                    lhs1 = fnl_t[:, k * 162 + 128: (k + 1) * 162]
                    nc.tensor.matmul(sims[0][:, :], lhs0, rhs,
                                     start=(k == 0), stop=(k == KC - 1))
                    nc.tensor.matmul(sims[1][:34, :], lhs1, rhs,
                                     start=(k == 0), stop=(k == KC - 1))

                for q in (0, 1):
                    sq = SQ[q]
                    e = sexp.tile([128, 324], f32, tag=f"e{j}{q}")
                    nc.scalar.activation(e[:sq, :], sims[q][:sq, :], AF.Exp,
                                         scale=TEMP)
                    exp_tiles[(j, q)] = e
                    # S row: lhsT = e_j ones column block  [sq, 8]
                    lsl = eones_t[:sq, q * 64 + j * 8: q * 64 + (j + 1) * 8]
                    nc.tensor.matmul(prow[:, 0:324], lsl, e[:sq, :],
                                     start=(j == 0 and q == 0), stop=False,
                                     skip_group_check=True)
                    # Z row (pos half only)
                    lml = emask_t[:sq, q * 64 + j * 8: q * 64 + (j + 1) * 8]
                    nc.tensor.matmul(prow[:, 324:486], lml, e[:sq, 0:162],
                                     start=False, stop=(j == 7 and q == 1),
                                     skip_group_check=True)

            # ---------------- row ops (batched over samples) ----------------
            srow_sb = srows.tile([8, 486], f32)
            nc.vector.tensor_copy(srow_sb[:], prow[:])
            recip = srows.tile([8, 324], f32)
            nc.vector.reciprocal(recip[:], srow_sb[:, 0:324])
            arow = srows.tile([8, 324], f32)
            nc.vector.tensor_mul(arow[:], recip[:], marow_t[:])
            nc.sync.dma_start(srow_o[:], srow_sb[:, 0:324])
            nc.sync.dma_start(zrow_o[:], srow_sb[:, 324:486])

            # ---------------- stage C: fold + recon ----------------
            for j in range(8):
                abc = pabc.tile([128, 324], f32, tag="abc")
                nc.tensor.matmul(abc[:], erow_t[:, j * 128:(j + 1) * 128],
                                 arow[:, :], start=True, stop=True)

                ft0 = sft.tile([128, C], f32r, tag="ft0")
                nc.sync.dma_start(ft0[:], ftq0[j])
                ft1 = sft.tile([128, C], f32r, tag="ft1")
                nc.sync.dma_start(ft1[:34, :], ftq1[j])
                dt0 = sft.tile([128, 324], f32, tag="dt0")
                nc.sync.dma_start(dt0[:], dtq0[j])
                dt1 = sft.tile([128, 324], f32, tag="dt1")
                nc.sync.dma_start(dt1[:34, :], dtq1[j])

                mps = []
                for q, dtl in ((0, dt0), (1, dt1)):
                    sq = SQ[q]
                    e = exp_tiles[(j, q)]
                    mpa = smp.tile([128, 324], f32, tag="mpa")
                    nc.vector.tensor_mul(mpa[:sq, :], e[:sq, :], abc[:sq, :])
                    mp = smp.tile([128, 324], f32r, tag=f"mp{q}")
                    nc.vector.tensor_add(mp[:sq, :], mpa[:sq, :], dtl[:sq, :])
                    mps.append(mp)

                rp_sb = sout.tile([128, KC * 162], f32, tag="rp")
                rn_sb = sout.tile([128, KC * 162], bf16, tag="rn")
                for ct in range(KC):
                    rp = prec.tile([128, 324], f32, tag="rec")
                    nc.tensor.matmul(rp[:], ft0[:, ct * 128:(ct + 1) * 128],
                                     mps[0][:, :], start=True, stop=False)
                    nc.tensor.matmul(rp[:], ft1[:34, ct * 128:(ct + 1) * 128],
                                     mps[1][:34, :], start=False, stop=True)
                    nc.scalar.copy(rp_sb[:, ct * 162:(ct + 1) * 162], rp[:, 0:162])
                    nc.vector.tensor_copy(rn_sb[:, ct * 162:(ct + 1) * 162],
                                          rp[:, 162:324])
                nc.sync.dma_start(recpos[j], rp_sb[:])
                nc.sync.dma_start(recneg[j], rn_sb[:])

    nc.compile()
    return nc


def _erow():
    e = np.zeros((8, 1024), np.float32)
    for j in range(8):
        e[j, j * 128:(j + 1) * 128] = 1.0
    return e


_NC_CACHE = None


def _get_nc():
    global _NC_CACHE
    if _NC_CACHE is None:
        _NC_CACHE = _build_bass()
    return _NC_CACHE


def prepare_in_maps(feat_v, feat_t, pos_idx, neg_idx):
    feat_v = np.asarray(feat_v, dtype=np.float32)
    feat_t = np.asarray(feat_t, dtype=np.float32)
    pos_idx = np.asarray(pos_idx).astype(np.int64)
    neg_idx = np.asarray(neg_idx).astype(np.int64)

    feat = np.concatenate([feat_v, feat_t], axis=0)       # [64, C, H, W]
    f = np.ascontiguousarray(feat.reshape(N, C, HW))      # [64, C, HW]

    norms = np.sqrt(np.einsum("ncs,ncs->ns", f, f, optimize=True))  # [N, HW]
    fn = f / np.maximum(norms, EPS_NORM)[:, None, :]
    mm_ = norms - norms.min(axis=1, keepdims=True)
    mask = mm_ / (mm_.max(axis=1, keepdims=True) + EPS_NORM)        # [N, HW]

    # fn tiled [N, KC, 128, HW] view for packing
    fn_t = fn.reshape(N, KC, 128, HW)

    in_maps = []
    octs = []
    for b in range(B):
        oct_idx = np.array(list(range(4 * b, 4 * b + 4))
                           + list(range(32 + 4 * b, 32 + 4 * b + 4)))
        octs.append(oct_idx)
        pos_g = pos_idx[oct_idx]           # global ids, inside octet
        neg_g = neg_idx[oct_idx]

        # fn_l: [8, 128, KC*162]  (j, p, k*162+t)
        fnl = np.ascontiguousarray(
            fn_t[oct_idx].transpose(0, 2, 1, 3).reshape(8, 128, KC * 162))
        # fnrhs: [8, 128, KC*324]
        fnr = np.empty((8, 128, KC, 324), np.float32)
        fnr[:, :, :, 0:162] = fn_t[pos_g].transpose(0, 2, 1, 3)
        fnr[:, :, :, 162:324] = fn_t[neg_g].transpose(0, 2, 1, 3)
        fnr = np.ascontiguousarray(fnr.reshape(8, 128, KC * 324))

        fT = np.ascontiguousarray(f[oct_idx].transpose(0, 2, 1))  # [8, HW, C]
        ft0 = np.ascontiguousarray(fT[:, 0:128, :])
        ft1 = np.ascontiguousarray(fT[:, 128:162, :])

        mk = mask[oct_idx]                                # [8, HW]
        dt_full = np.zeros((8, HW, 324), np.float32)
        one_minus = 1.0 - mk                              # [8, HW]
        srange = np.arange(HW)
        dt_full[:, srange, srange] = one_minus
        dt_full[:, srange, 162 + srange] = one_minus
        dtq0 = np.ascontiguousarray(dt_full[:, 0:128, :])
        dtq1 = np.ascontiguousarray(dt_full[:, 128:162, :])

        eones = np.zeros((128, 2, 8, 8), np.float32)
        emask = np.zeros((128, 2, 8, 8), np.float32)
        for q in (0, 1):
            sq = SQ[q]
            for j in range(8):
                eones[:sq, q, j, j] = 1.0
                emask[:sq, q, j, j] = mk[j, q * 128: q * 128 + sq]
        eones = eones.reshape(128, 128)
        emask = emask.reshape(128, 128)

        marow = np.concatenate([mk, mk], axis=1)          # [8, 324]

        in_maps.append({
            "fn_l": fnl, "fnrhs": fnr,
            "ftq0": ft0, "ftq1": ft1,
            "dtq0": dtq0, "dtq1": dtq1,
            "eones": np.ascontiguousarray(eones),
            "emask": np.ascontiguousarray(emask),
            "erow": _erow(),
            "marow": np.ascontiguousarray(marow),
        })

    return {"in_maps": in_maps, "octs": octs, "f": f, "mask": mask,
            "pos_idx": pos_idx, "neg_idx": neg_idx}


def kernel(feat_v, feat_t, pos_idx, neg_idx):
    from concourse import bass_utils

    prep = prepare_in_maps(feat_v, feat_t, pos_idx, neg_idx)
    in_maps, octs, f, mask = (prep["in_maps"], prep["octs"],
                              prep["f"], prep["mask"])
    pos_idx = prep["pos_idx"]

    nc = _get_nc()
    res = bass_utils.run_bass_kernel_spmd(
        nc, in_maps, core_ids=list(range(NCORES)), trace=TRACE)
    LAST["exec_time_ns"] = res.exec_time_ns
    LAST["mean_exec_time_ns"] = res.mean_exec_time_ns
    LAST["trace"] = res.instructions_and_trace[1] if res.instructions_and_trace else None

    recon_pos = np.zeros((N, C, HW), np.float32)
    recon_neg = np.zeros((N, C, HW), np.float32)
    S_pos = np.zeros((N, 162), np.float32)
    Z = np.zeros((N, 162), np.float32)
    for b in range(B):
        r_ = res.results[b]
        oct_idx = octs[b]
        rp = r_["recpos"].reshape(8, 128, KC, 162).transpose(0, 2, 1, 3)
        recon_pos[oct_idx] = rp.reshape(8, C, HW)
        rn = np.asarray(r_["recneg"], dtype=np.float32)
        rn = rn.reshape(8, 128, KC, 162).transpose(0, 2, 1, 3)
        recon_neg[oct_idx] = rn.reshape(8, C, HW)
        S_pos[oct_idx] = r_["srow"][:, 0:162]
        Z[oct_idx] = r_["zrow"]

    # host epilogue: distances, comask, loss
    d_ap = np.sqrt(((f - recon_pos + EPS_DIST) ** 2).sum(axis=1))   # [N, HW]
    d_an = np.sqrt(((f - recon_neg + EPS_DIST) ** 2).sum(axis=1))
    trip = np.maximum(d_ap - d_an + MARGIN, 0.0)

    mask_warp = Z / S_pos
    comask = mask_warp * mask[pos_idx]
    comask = comask - comask.min(axis=1, keepdims=True)
    comask = comask / (comask.max(axis=1, keepdims=True) + EPS_NORM)

    loss = np.float32((comask.sum(axis=0) * trip.sum(axis=0)).sum()
                      / (N * N * HW))
    recon = recon_pos.reshape(N, C, H, W)
    return recon, loss


# revision 15
# speedup vs baseline: 278.8681x; 278.8681x over previous
"""CMAlign Trainium2 kernel (Bass/Tile, 8-core SPMD).

Sharding: one identity octet per core (samples {4b..4b+3, 32+4b..32+4b+3}).
pos pairs are octet-internal; neg rows gathered on host.

Device (per sample j):
  simT[s,t] = sum_c fn[j][c,s] * fn[pair][c,t]   (fp32r matmuls, rhs packs pos|neg, N=324)
  expT = exp(50*simT)                            (ScalarE)
  S[t] = sum_s expT  (ones-matmul, e_j-packed)   Z[t] = sum_s mask[s]*expT_pos
  A = mask/S (DVE, batched rows);  mp' = A*expT + (1-mask[t])*delta[s,t]
  recon[c,t] = sum_s fT_raw[s,c] * mp'[s,t]      -> PSUM [recon_pos | recon_neg]
  evac: pos fp32 (ScalarE), neg bf16 (VectorE)

Host: norms/masks/normalize + gathers (pre), distances/trip/comask/loss (post).
"""

import os
import numpy as np

B, P, C, H, W = 8, 4, 2048, 18, 9
N = 2 * B * P            # 64
HW = H * W               # 162
KC = C // 128            # 16 channel tiles
SQ = (128, 34)           # s-chunk sizes
TEMP = 50.0
MARGIN = 0.3
EPS_NORM = 1e-12
EPS_DIST = 1e-6
NCORES = 8

TRACE = False            # set True (e.g. from test.py) to profile
LAST = {}                # exec_time_ns etc. stashed here after a run


def _build_bass(loop_iters=1):
    import contextlib
    import concourse.bacc as bacc
    import concourse.tile as tile
    import concourse.mybir as mybir

    dt = mybir.dt
    f32 = dt.float32
    f32r = dt.float32r
    bf16 = dt.bfloat16
    AF = mybir.ActivationFunctionType

    nc = bacc.Bacc("TRN2", target_bir_lowering=False, debug=False,
                   num_devices=NCORES)

    # ---- DRAM I/O (per-core shapes) ----
    fn_l = nc.dram_tensor("fn_l", [8, 128, KC * 162], f32r, kind="ExternalInput").ap()
    fnrhs = nc.dram_tensor("fnrhs", [8, 128, KC * 324], f32r, kind="ExternalInput").ap()
    ftq0 = nc.dram_tensor("ftq0", [8, 128, C], f32r, kind="ExternalInput").ap()
    ftq1 = nc.dram_tensor("ftq1", [8, 34, C], f32r, kind="ExternalInput").ap()
    dtq0 = nc.dram_tensor("dtq0", [8, 128, 324], f32, kind="ExternalInput").ap()
    dtq1 = nc.dram_tensor("dtq1", [8, 34, 324], f32, kind="ExternalInput").ap()
    eones = nc.dram_tensor("eones", [128, 128], f32, kind="ExternalInput").ap()
    emask = nc.dram_tensor("emask", [128, 128], f32, kind="ExternalInput").ap()
    erow = nc.dram_tensor("erow", [8, 1024], f32, kind="ExternalInput").ap()
    marow = nc.dram_tensor("marow", [8, 324], f32, kind="ExternalInput").ap()

    recpos = nc.dram_tensor("recpos", [8, 128, KC * 162], f32, kind="ExternalOutput").ap()
    recneg = nc.dram_tensor("recneg", [8, 128, KC * 162], bf16, kind="ExternalOutput").ap()
    srow_o = nc.dram_tensor("srow", [8, 324], f32, kind="ExternalOutput").ap()
    zrow_o = nc.dram_tensor("zrow", [8, 162], f32, kind="ExternalOutput").ap()

    def r(ap):
        return ap.bitcast(f32r)

    with tile.TileContext(nc) as tc:
        with (
            tc.tile_pool(name="const", bufs=1) as const,
            tc.tile_pool(name="sin", bufs=2) as sin,
            tc.tile_pool(name="sexp", bufs=1) as sexp,
            tc.tile_pool(name="sft", bufs=2) as sft,
            tc.tile_pool(name="smp", bufs=2) as smp,
            tc.tile_pool(name="sout", bufs=2) as sout,
            tc.tile_pool(name="srows", bufs=1) as srows,
            tc.tile_pool(name="psim", bufs=2, space="PSUM") as psim,
            tc.tile_pool(name="prec", bufs=2, space="PSUM") as prec,
            tc.tile_pool(name="prow", bufs=1, space="PSUM") as prow_pool,
            tc.tile_pool(name="pabc", bufs=1, space="PSUM") as pabc,
        ):
            # constants
            eones_t = const.tile([128, 128], f32)
            nc.sync.dma_start(eones_t[:], eones[:])
            emask_t = const.tile([128, 128], f32)
            nc.sync.dma_start(emask_t[:], emask[:])
            erow_t = const.tile([8, 1024], f32)
            nc.sync.dma_start(erow_t[:], erow[:])
            marow_t = const.tile([8, 324], f32)
            nc.sync.dma_start(marow_t[:], marow[:])

            prow = prow_pool.tile([8, 486], f32)  # cols 0:324 = S, 324:486 = Z

            loop_cm = (tc.For_i(0, loop_iters, 1) if loop_iters > 1
                       else contextlib.nullcontext())
            with loop_cm:
                _emit_body(nc, tc, mybir, locals())
    nc.compile()
    return nc


def _emit_body(nc, tc, mybir, env):
    dt = mybir.dt
    f32 = dt.float32
    f32r = dt.float32r
    bf16 = dt.bfloat16
    AF = mybir.ActivationFunctionType
    (fn_l, fnrhs, ftq0, ftq1, dtq0, dtq1, recpos, recneg, srow_o, zrow_o,
     eones_t, emask_t, erow_t, marow_t, prow, sin, sexp, sft, smp, sout,
     srows, psim, prec, pabc) = (
        env["fn_l"], env["fnrhs"], env["ftq0"], env["ftq1"], env["dtq0"],
        env["dtq1"], env["recpos"], env["recneg"], env["srow_o"],
        env["zrow_o"], env["eones_t"], env["emask_t"], env["erow_t"],
        env["marow_t"], env["prow"], env["sin"], env["sexp"], env["sft"],
        env["smp"], env["sout"], env["srows"], env["psim"], env["prec"],
        env["pabc"])
    if True:
        if True:
            exp_tiles = {}

            # ---------------- stage A: sim + exp + S/Z rows ----------------
            for j in range(8):
                fnl_t = sin.tile([128, KC * 162], f32r, tag="fnl")
                nc.sync.dma_start(fnl_t[:], fn_l[j])
                fnr_t = sin.tile([128, KC * 324], f32r, tag="fnr")
                nc.sync.dma_start(fnr_t[:], fnrhs[j])

                sims = [psim.tile([128, 324], f32, tag="sim0", name="sim0"),
                        psim.tile([128, 324], f32, tag="sim1", name="sim1")]
                for k in range(KC):
                    rhs = fnr_t[:, k * 324:(k + 1) * 324]
                    lhs0 = fnl_t[:, k * 162: k * 162 + 128]
                    lhs1 = fnl_t[:, k * 162 + 128: (k + 1) * 162]
                    nc.tensor.matmul(sims[0][:, :], lhs0, rhs,
                                     start=(k == 0), stop=(k == KC - 1))
                    nc.tensor.matmul(sims[1][:34, :], lhs1, rhs,
                                     start=(k == 0), stop=(k == KC - 1))

                for q in (0, 1):
                    sq = SQ[q]
                    e = sexp.tile([128, 324], f32, tag=f"e{j}{q}")
                    nc.scalar.activation(e[:sq, :], sims[q][:sq, :], AF.Exp,
                                         scale=TEMP)
                    exp_tiles[(j, q)] = e
                    # S row: lhsT = e_j ones column block  [sq, 8]
                    lsl = eones_t[:sq, q * 64 + j * 8: q * 64 + (j + 1) * 8]
                    nc.tensor.matmul(prow[:, 0:324], lsl, e[:sq, :],
                                     start=(j == 0 and q == 0), stop=False,
                                     skip_group_check=True)
                    # Z row (pos half only)
                    lml = emask_t[:sq, q * 64 + j * 8: q * 64 + (j + 1) * 8]
                    nc.tensor.matmul(prow[:, 324:486], lml, e[:sq, 0:162],
                                     start=False, stop=(j == 7 and q == 1),
                                     skip_group_check=True)

            # ---------------- row ops (batched over samples) ----------------
            srow_sb = srows.tile([8, 486], f32)
            nc.vector.tensor_copy(srow_sb[:], prow[:])
            recip = srows.tile([8, 324], f32)
            nc.vector.reciprocal(recip[:], srow_sb[:, 0:324])
            arow = srows.tile([8, 324], f32)
            nc.vector.tensor_mul(arow[:], recip[:], marow_t[:])
            nc.sync.dma_start(srow_o[:], srow_sb[:, 0:324])
            nc.sync.dma_start(zrow_o[:], srow_sb[:, 324:486])

            # ---------------- stage C: fold + recon ----------------
            for j in range(8):
                abc = pabc.tile([128, 324], f32, tag="abc")
                nc.tensor.matmul(abc[:], erow_t[:, j * 128:(j + 1) * 128],
                                 arow[:, :], start=True, stop=True)

                ft0 = sft.tile([128, C], f32r, tag="ft0")
                nc.sync.dma_start(ft0[:], ftq0[j])
                ft1 = sft.tile([128, C], f32r, tag="ft1")
                nc.sync.dma_start(ft1[:34, :], ftq1[j])
                dt0 = sft.tile([128, 324], f32, tag="dt0")
                nc.sync.dma_start(dt0[:], dtq0[j])
                dt1 = sft.tile([128, 324], f32, tag="dt1")
                nc.sync.dma_start(dt1[:34, :], dtq1[j])

                mps = []
                for q, dtl in ((0, dt0), (1, dt1)):
                    sq = SQ[q]
                    e = exp_tiles[(j, q)]
                    mpa = smp.tile([128, 324], f32, tag="mpa")
                    nc.vector.tensor_mul(mpa[:sq, :], e[:sq, :], abc[:sq, :])
                    mp = smp.tile([128, 324], f32r, tag=f"mp{q}")
                    nc.vector.tensor_add(mp[:sq, :], mpa[:sq, :], dtl[:sq, :])
                    mps.append(mp)

                rp_sb = sout.tile([128, KC * 162], f32, tag="rp")
                rn_sb = sout.tile([128, KC * 162], bf16, tag="rn")
                for ct in range(KC):
                    rp = prec.tile([128, 324], f32, tag="rec")
                    nc.tensor.matmul(rp[:], ft0[:, ct * 128:(ct + 1) * 128],
                                     mps[0][:, :], start=True, stop=False)
                    nc.tensor.matmul(rp[:], ft1[:34, ct * 128:(ct + 1) * 128],
                                     mps[1][:34, :], start=False, stop=True)
                    nc.scalar.copy(rp_sb[:, ct * 162:(ct + 1) * 162], rp[:, 0:162])
                    nc.vector.tensor_copy(rn_sb[:, ct * 162:(ct + 1) * 162],
                                          rp[:, 162:324])
                nc.sync.dma_start(recpos[j], rp_sb[:])
                nc.sync.dma_start(recneg[j], rn_sb[:])


def _erow():
    e = np.zeros((8, 1024), np.float32)
    for j in range(8):
        e[j, j * 128:(j + 1) * 128] = 1.0
    return e


_NC_CACHE = None


def _get_nc():
    global _NC_CACHE
    if _NC_CACHE is None:
        _NC_CACHE = _build_bass()
    return _NC_CACHE


def prepare_in_maps(feat_v, feat_t, pos_idx, neg_idx):
    feat_v = np.asarray(feat_v, dtype=np.float32)
    feat_t = np.asarray(feat_t, dtype=np.float32)
    pos_idx = np.asarray(pos_idx).astype(np.int64)
    neg_idx = np.asarray(neg_idx).astype(np.int64)

    feat = np.concatenate([feat_v, feat_t], axis=0)       # [64, C, H, W]
    f = np.ascontiguousarray(feat.reshape(N, C, HW))      # [64, C, HW]

    norms = np.sqrt(np.einsum("ncs,ncs->ns", f, f, optimize=True))  # [N, HW]
    fn = f / np.maximum(norms, EPS_NORM)[:, None, :]
    mm_ = norms - norms.min(axis=1, keepdims=True)
    mask = mm_ / (mm_.max(axis=1, keepdims=True) + EPS_NORM)        # [N, HW]

    # fn tiled [N, KC, 128, HW] view for packing
    fn_t = fn.reshape(N, KC, 128, HW)

    in_maps = []
    octs = []
    for b in range(B):
        oct_idx = np.array(list(range(4 * b, 4 * b + 4))
                           + list(range(32 + 4 * b, 32 + 4 * b + 4)))
        octs.append(oct_idx)
        pos_g = pos_idx[oct_idx]           # global ids, inside octet
        neg_g = neg_idx[oct_idx]

        # fn_l: [8, 128, KC*162]  (j, p, k*162+t)
        fnl = np.ascontiguousarray(
            fn_t[oct_idx].transpose(0, 2, 1, 3).reshape(8, 128, KC * 162))
        # fnrhs: [8, 128, KC*324]
        fnr = np.empty((8, 128, KC, 324), np.float32)
        fnr[:, :, :, 0:162] = fn_t[pos_g].transpose(0, 2, 1, 3)
        fnr[:, :, :, 162:324] = fn_t[neg_g].transpose(0, 2, 1, 3)
        fnr = np.ascontiguousarray(fnr.reshape(8, 128, KC * 324))

        fT = np.ascontiguousarray(f[oct_idx].transpose(0, 2, 1))  # [8, HW, C]
        ft0 = np.ascontiguousarray(fT[:, 0:128, :])
        ft1 = np.ascontiguousarray(fT[:, 128:162, :])

        mk = mask[oct_idx]                                # [8, HW]
        dt_full = np.zeros((8, HW, 324), np.float32)
        one_minus = 1.0 - mk                              # [8, HW]
        srange = np.arange(HW)
        dt_full[:, srange, srange] = one_minus
        dt_full[:, srange, 162 + srange] = one_minus
        dtq0 = np.ascontiguousarray(dt_full[:, 0:128, :])
        dtq1 = np.ascontiguousarray(dt_full[:, 128:162, :])

        eones = np.zeros((128, 2, 8, 8), np.float32)
        emask = np.zeros((128, 2, 8, 8), np.float32)
        for q in (0, 1):
            sq = SQ[q]
            for j in range(8):
                eones[:sq, q, j, j] = 1.0
                emask[:sq, q, j, j] = mk[j, q * 128: q * 128 + sq]
        eones = eones.reshape(128, 128)
        emask = emask.reshape(128, 128)

        marow = np.concatenate([mk, mk], axis=1)          # [8, 324]

        in_maps.append({
            "fn_l": fnl, "fnrhs": fnr,
            "ftq0": ft0, "ftq1": ft1,
            "dtq0": dtq0, "dtq1": dtq1,
            "eones": np.ascontiguousarray(eones),
            "emask": np.ascontiguousarray(emask),
            "erow": _erow(),
            "marow": np.ascontiguousarray(marow),
        })

    return {"in_maps": in_maps, "octs": octs, "f": f, "mask": mask,
            "pos_idx": pos_idx, "neg_idx": neg_idx}


def kernel(feat_v, feat_t, pos_idx, neg_idx):
    from concourse import bass_utils

    prep = prepare_in_maps(feat_v, feat_t, pos_idx, neg_idx)
    in_maps, octs, f, mask = (prep["in_maps"], prep["octs"],
                              prep["f"], prep["mask"])
    pos_idx = prep["pos_idx"]

    nc = _get_nc()
    res = bass_utils.run_bass_kernel_spmd(
        nc, in_maps, core_ids=list(range(NCORES)), trace=TRACE)
    LAST["exec_time_ns"] = res.exec_time_ns
    LAST["mean_exec_time_ns"] = res.mean_exec_time_ns
    LAST["trace"] = res.instructions_and_trace[1] if res.instructions_and_trace else None

    recon_pos = np.zeros((N, C, HW), np.float32)
    recon_neg = np.zeros((N, C, HW), np.float32)
    S_pos = np.zeros((N, 162), np.float32)
    Z = np.zeros((N, 162), np.float32)
    for b in range(B):
        r_ = res.results[b]
        oct_idx = octs[b]
        rp = r_["recpos"].reshape(8, 128, KC, 162).transpose(0, 2, 1, 3)
        recon_pos[oct_idx] = rp.reshape(8, C, HW)
        rn = np.asarray(r_["recneg"], dtype=np.float32)
        rn = rn.reshape(8, 128, KC, 162).transpose(0, 2, 1, 3)
        recon_neg[oct_idx] = rn.reshape(8, C, HW)
        S_pos[oct_idx] = r_["srow"][:, 0:162]
        Z[oct_idx] = r_["zrow"]

    # host epilogue: distances, comask, loss
    d_ap = np.sqrt(((f - recon_pos + EPS_DIST) ** 2).sum(axis=1))   # [N, HW]
    d_an = np.sqrt(((f - recon_neg + EPS_DIST) ** 2).sum(axis=1))
    trip = np.maximum(d_ap - d_an + MARGIN, 0.0)

    mask_warp = Z / S_pos
    comask = mask_warp * mask[pos_idx]
    comask = comask - comask.min(axis=1, keepdims=True)
    comask = comask / (comask.max(axis=1, keepdims=True) + EPS_NORM)

    loss = np.float32((comask.sum(axis=0) * trip.sum(axis=0)).sum()
                      / (N * N * HW))
    recon = recon_pos.reshape(N, C, H, W)
    return recon, loss


# revision 17
# speedup vs baseline: 487.0678x; 1.7466x over previous
"""CMAlign Trainium2 kernel (Bass/Tile, 8-core SPMD).

Sharding: one identity octet per core (samples {4b..4b+3, 32+4b..32+4b+3}).
pos pairs are octet-internal; neg rows gathered on host.

Device (per sample j):
  simT[s,t] = sum_c fn[j][c,s] * fn[pair][c,t]   (fp32r matmuls, rhs packs pos|neg, N=324)
  expT = exp(50*simT)                            (ScalarE)
  S[t] = sum_s expT  (ones-matmul, e_j-packed)   Z[t] = sum_s mask[s]*expT_pos
  A = mask/S (DVE, batched rows);  mp' = A*expT + (1-mask[t])*delta[s,t]
  recon[c,t] = sum_s fT_raw[s,c] * mp'[s,t]      -> PSUM [recon_pos | recon_neg]
  evac: pos fp32 (ScalarE), neg bf16 (VectorE)

Host: norms/masks/normalize + gathers (pre), distances/trip/comask/loss (post).
"""

import os
import numpy as np

B, P, C, H, W = 8, 4, 2048, 18, 9
N = 2 * B * P            # 64
HW = H * W               # 162
KC = C // 128            # 16 channel tiles
SQ = (128, 34)           # s-chunk sizes
TEMP = 50.0
MARGIN = 0.3
EPS_NORM = 1e-12
EPS_DIST = 1e-6
NCORES = 8

TRACE = False            # set True (e.g. from test.py) to profile
LAST = {}                # exec_time_ns etc. stashed here after a run


def _build_bass(loop_iters=1):
    import contextlib
    import concourse.bacc as bacc
    import concourse.tile as tile
    import concourse.mybir as mybir

    dt = mybir.dt
    f32 = dt.float32
    f32r = dt.float32r
    bf16 = dt.bfloat16
    AF = mybir.ActivationFunctionType

    nc = bacc.Bacc("TRN2", target_bir_lowering=False, debug=False,
                   num_devices=NCORES)

    # ---- DRAM I/O (per-core shapes) ----
    fn_l = nc.dram_tensor("fn_l", [8, 128, KC * 162], f32r, kind="ExternalInput").ap()
    fnrhs = nc.dram_tensor("fnrhs", [8, 128, KC * 324], f32r, kind="ExternalInput").ap()
    ftq0 = nc.dram_tensor("ftq0", [8, 128, C], bf16, kind="ExternalInput").ap()
    ftq1 = nc.dram_tensor("ftq1", [8, 34, C], bf16, kind="ExternalInput").ap()
    dtq0 = nc.dram_tensor("dtq0", [8, 128, 324], f32, kind="ExternalInput").ap()
    dtq1 = nc.dram_tensor("dtq1", [8, 34, 324], f32, kind="ExternalInput").ap()
    eones = nc.dram_tensor("eones", [128, 128], f32, kind="ExternalInput").ap()
    emask = nc.dram_tensor("emask", [128, 128], f32, kind="ExternalInput").ap()
    erow = nc.dram_tensor("erow", [8, 1024], f32, kind="ExternalInput").ap()
    marow = nc.dram_tensor("marow", [8, 324], f32, kind="ExternalInput").ap()

    recpos = nc.dram_tensor("recpos", [8, 128, KC * 162], f32, kind="ExternalOutput").ap()
    recneg = nc.dram_tensor("recneg", [8, 128, KC * 162], bf16, kind="ExternalOutput").ap()
    srow_o = nc.dram_tensor("srow", [8, 324], f32, kind="ExternalOutput").ap()
    zrow_o = nc.dram_tensor("zrow", [8, 162], f32, kind="ExternalOutput").ap()

    def r(ap):
        return ap.bitcast(f32r)

    with tile.TileContext(nc) as tc:
        with (
            tc.tile_pool(name="const", bufs=1) as const,
            tc.tile_pool(name="sin", bufs=2) as sin,
            tc.tile_pool(name="sexp", bufs=1) as sexp,
            tc.tile_pool(name="sft", bufs=2) as sft,
            tc.tile_pool(name="smp", bufs=2) as smp,
            tc.tile_pool(name="sout", bufs=2) as sout,
            tc.tile_pool(name="srows", bufs=1) as srows,
            tc.tile_pool(name="psim", bufs=2, space="PSUM") as psim,
            tc.tile_pool(name="prec", bufs=2, space="PSUM") as prec,
            tc.tile_pool(name="prow", bufs=1, space="PSUM") as prow_pool,
            tc.tile_pool(name="pabc", bufs=1, space="PSUM") as pabc,
        ):
            # constants
            eones_t = const.tile([128, 128], f32)
            nc.sync.dma_start(eones_t[:], eones[:])
            emask_t = const.tile([128, 128], f32)
            nc.sync.dma_start(emask_t[:], emask[:])
            erow_t = const.tile([8, 1024], f32)
            nc.sync.dma_start(erow_t[:], erow[:])
            marow_t = const.tile([8, 324], f32)
            nc.sync.dma_start(marow_t[:], marow[:])

            prow = prow_pool.tile([8, 486], f32)  # cols 0:324 = S, 324:486 = Z

            loop_cm = (tc.For_i(0, loop_iters, 1) if loop_iters > 1
                       else contextlib.nullcontext())
            with loop_cm:
                _emit_body(nc, tc, mybir, locals())
    nc.compile()
    return nc


def _emit_body(nc, tc, mybir, env):
    dt = mybir.dt
    f32 = dt.float32
    f32r = dt.float32r
    bf16 = dt.bfloat16
    AF = mybir.ActivationFunctionType
    (fn_l, fnrhs, ftq0, ftq1, dtq0, dtq1, recpos, recneg, srow_o, zrow_o,
     eones_t, emask_t, erow_t, marow_t, prow, sin, sexp, sft, smp, sout,
     srows, psim, prec, pabc) = (
        env["fn_l"], env["fnrhs"], env["ftq0"], env["ftq1"], env["dtq0"],
        env["dtq1"], env["recpos"], env["recneg"], env["srow_o"],
        env["zrow_o"], env["eones_t"], env["emask_t"], env["erow_t"],
        env["marow_t"], env["prow"], env["sin"], env["sexp"], env["sft"],
        env["smp"], env["sout"], env["srows"], env["psim"], env["prec"],
        env["pabc"])
    if True:
        if True:
            exp_tiles = {}

            # ---------------- stage A: sim + exp + S/Z rows ----------------
            for j in range(8):
                fnl_t = sin.tile([128, KC * 162], f32r, tag="fnl")
                H1 = KC * 162 // 2
                nc.sync.dma_start(fnl_t[:, 0:H1], fn_l[j][:, 0:H1])
                nc.sync.dma_start(fnl_t[:, H1:], fn_l[j][:, H1:])
                fnr_t = sin.tile([128, KC * 324], f32r, tag="fnr")
                T3 = KC * 324 // 4
                for u in range(4):
                    nc.sync.dma_start(fnr_t[:, u * T3:(u + 1) * T3],
                                      fnrhs[j][:, u * T3:(u + 1) * T3])

                sims = [psim.tile([128, 324], f32, tag="sim0", name="sim0"),
                        psim.tile([128, 324], f32, tag="sim1", name="sim1")]
                for k in range(KC):
                    rhs = fnr_t[:, k * 324:(k + 1) * 324]
                    lhs0 = fnl_t[:, k * 162: k * 162 + 128]
                    lhs1 = fnl_t[:, k * 162 + 128: (k + 1) * 162]
                    nc.tensor.matmul(sims[0][:, :], lhs0, rhs,
                                     start=(k == 0), stop=(k == KC - 1))
                    nc.tensor.matmul(sims[1][:34, :], lhs1, rhs,
                                     start=(k == 0), stop=(k == KC - 1))

                for q in (0, 1):
                    sq = SQ[q]
                    e = sexp.tile([128, 324], f32, tag=f"e{j}{q}")
                    nc.scalar.activation(e[:sq, :], sims[q][:sq, :], AF.Exp,
                                         scale=TEMP)
                    exp_tiles[(j, q)] = e
                    # S row: lhsT = e_j ones column block  [sq, 8]
                    lsl = eones_t[:sq, q * 64 + j * 8: q * 64 + (j + 1) * 8]
                    nc.tensor.matmul(prow[:, 0:324], lsl, e[:sq, :],
                                     start=(j == 0 and q == 0), stop=False,
                                     skip_group_check=True)
                    # Z row (pos half only)
                    lml = emask_t[:sq, q * 64 + j * 8: q * 64 + (j + 1) * 8]
                    nc.tensor.matmul(prow[:, 324:486], lml, e[:sq, 0:162],
                                     start=False, stop=(j == 7 and q == 1),
                                     skip_group_check=True)

            # ---------------- row ops (batched over samples) ----------------
            srow_sb = srows.tile([8, 486], f32)
            nc.vector.tensor_copy(srow_sb[:], prow[:])
            recip = srows.tile([8, 324], f32)
            nc.vector.reciprocal(recip[:], srow_sb[:, 0:324])
            arow = srows.tile([8, 324], f32)
            nc.vector.tensor_mul(arow[:], recip[:], marow_t[:])
            nc.sync.dma_start(srow_o[:], srow_sb[:, 0:324])
            nc.sync.dma_start(zrow_o[:], srow_sb[:, 324:486])

            # ---------------- stage C: fold + recon ----------------
            for j in range(8):
                abc = pabc.tile([128, 324], f32, tag="abc")
                nc.tensor.matmul(abc[:], erow_t[:, j * 128:(j + 1) * 128],
                                 arow[:, :], start=True, stop=True)

                ft0 = sft.tile([128, C], bf16, tag="ft0")
                nc.sync.dma_start(ft0[:, 0:C // 2], ftq0[j][:, 0:C // 2])
                nc.sync.dma_start(ft0[:, C // 2:], ftq0[j][:, C // 2:])
                ft1 = sft.tile([128, C], bf16, tag="ft1")
                nc.sync.dma_start(ft1[:34, :], ftq1[j])
                dt0 = sft.tile([128, 324], f32, tag="dt0")
                nc.sync.dma_start(dt0[:], dtq0[j])
                dt1 = sft.tile([128, 324], f32, tag="dt1")
                nc.sync.dma_start(dt1[:34, :], dtq1[j])

                mps = []
                for q, dtl in ((0, dt0), (1, dt1)):
                    sq = SQ[q]
                    e = exp_tiles[(j, q)]
                    mpa = smp.tile([128, 324], f32, tag="mpa")
                    nc.vector.tensor_mul(mpa[:sq, :], e[:sq, :], abc[:sq, :])
                    mp = smp.tile([128, 324], bf16, tag=f"mp{q}")
                    nc.vector.tensor_add(mp[:sq, :], mpa[:sq, :], dtl[:sq, :])
                    mps.append(mp)

                rp_sb = sout.tile([128, KC * 162], f32, tag="rp")
                rn_sb = sout.tile([128, KC * 162], bf16, tag="rn")
                for ct in range(KC):
                    rp = prec.tile([128, 324], f32, tag="rec")
                    nc.tensor.matmul(rp[:], ft0[:, ct * 128:(ct + 1) * 128],
                                     mps[0][:, :], start=True, stop=False)
                    nc.tensor.matmul(rp[:], ft1[:34, ct * 128:(ct + 1) * 128],
                                     mps[1][:34, :], start=False, stop=True)
                    nc.scalar.copy(rp_sb[:, ct * 162:(ct + 1) * 162], rp[:, 0:162])
                    nc.vector.tensor_copy(rn_sb[:, ct * 162:(ct + 1) * 162],
                                          rp[:, 162:324])
                HP = KC * 162 // 2
                nc.sync.dma_start(recpos[j][:, 0:HP], rp_sb[:, 0:HP])
                nc.sync.dma_start(recpos[j][:, HP:], rp_sb[:, HP:])
                nc.sync.dma_start(recneg[j], rn_sb[:])


def _erow():
    e = np.zeros((8, 1024), np.float32)
    for j in range(8):
        e[j, j * 128:(j + 1) * 128] = 1.0
    return e


_NC_CACHE = None


def _get_nc():
    global _NC_CACHE
    if _NC_CACHE is None:
        _NC_CACHE = _build_bass()
    return _NC_CACHE


def prepare_in_maps(feat_v, feat_t, pos_idx, neg_idx):
    feat_v = np.asarray(feat_v, dtype=np.float32)
    feat_t = np.asarray(feat_t, dtype=np.float32)
    pos_idx = np.asarray(pos_idx).astype(np.int64)
    neg_idx = np.asarray(neg_idx).astype(np.int64)

    feat = np.concatenate([feat_v, feat_t], axis=0)       # [64, C, H, W]
    f = np.ascontiguousarray(feat.reshape(N, C, HW))      # [64, C, HW]

    norms = np.sqrt(np.einsum("ncs,ncs->ns", f, f, optimize=True))  # [N, HW]
    fn = f / np.maximum(norms, EPS_NORM)[:, None, :]
    mm_ = norms - norms.min(axis=1, keepdims=True)
    mask = mm_ / (mm_.max(axis=1, keepdims=True) + EPS_NORM)        # [N, HW]

    # fn tiled [N, KC, 128, HW] view for packing
    fn_t = fn.reshape(N, KC, 128, HW)

    in_maps = []
    octs = []
    for b in range(B):
        oct_idx = np.array(list(range(4 * b, 4 * b + 4))
                           + list(range(32 + 4 * b, 32 + 4 * b + 4)))
        octs.append(oct_idx)
        pos_g = pos_idx[oct_idx]           # global ids, inside octet
        neg_g = neg_idx[oct_idx]

        # fn_l: [8, 128, KC*162]  (j, p, k*162+t)
        fnl = np.ascontiguousarray(
            fn_t[oct_idx].transpose(0, 2, 1, 3).reshape(8, 128, KC * 162))
        # fnrhs: [8, 128, KC*324]
        fnr = np.empty((8, 128, KC, 324), np.float32)
        fnr[:, :, :, 0:162] = fn_t[pos_g].transpose(0, 2, 1, 3)
        fnr[:, :, :, 162:324] = fn_t[neg_g].transpose(0, 2, 1, 3)
        fnr = np.ascontiguousarray(fnr.reshape(8, 128, KC * 324))

        import ml_dtypes
        fT = np.ascontiguousarray(f[oct_idx].transpose(0, 2, 1))  # [8, HW, C]
        ft0 = np.ascontiguousarray(fT[:, 0:128, :]).astype(ml_dtypes.bfloat16)
        ft1 = np.ascontiguousarray(fT[:, 128:162, :]).astype(ml_dtypes.bfloat16)

        mk = mask[oct_idx]                                # [8, HW]
        dt_full = np.zeros((8, HW, 324), np.float32)
        one_minus = 1.0 - mk                              # [8, HW]
        srange = np.arange(HW)
        dt_full[:, srange, srange] = one_minus
        dt_full[:, srange, 162 + srange] = one_minus
        dtq0 = np.ascontiguousarray(dt_full[:, 0:128, :])
        dtq1 = np.ascontiguousarray(dt_full[:, 128:162, :])

        eones = np.zeros((128, 2, 8, 8), np.float32)
        emask = np.zeros((128, 2, 8, 8), np.float32)
        for q in (0, 1):
            sq = SQ[q]
            for j in range(8):
                eones[:sq, q, j, j] = 1.0
                emask[:sq, q, j, j] = mk[j, q * 128: q * 128 + sq]
        eones = eones.reshape(128, 128)
        emask = emask.reshape(128, 128)

        marow = np.concatenate([mk, mk], axis=1)          # [8, 324]

        in_maps.append({
            "fn_l": fnl, "fnrhs": fnr,
            "ftq0": ft0, "ftq1": ft1,
            "dtq0": dtq0, "dtq1": dtq1,
            "eones": np.ascontiguousarray(eones),
            "emask": np.ascontiguousarray(emask),
            "erow": _erow(),
            "marow": np.ascontiguousarray(marow),
        })

    return {"in_maps": in_maps, "octs": octs, "f": f, "mask": mask,
            "pos_idx": pos_idx, "neg_idx": neg_idx}


def kernel(feat_v, feat_t, pos_idx, neg_idx):
    from concourse import bass_utils

    prep = prepare_in_maps(feat_v, feat_t, pos_idx, neg_idx)
    in_maps, octs, f, mask = (prep["in_maps"], prep["octs"],
                              prep["f"], prep["mask"])
    pos_idx = prep["pos_idx"]

    nc = _get_nc()
    res = bass_utils.run_bass_kernel_spmd(
        nc, in_maps, core_ids=list(range(NCORES)), trace=TRACE)
    LAST["exec_time_ns"] = res.exec_time_ns
    LAST["mean_exec_time_ns"] = res.mean_exec_time_ns
    LAST["trace"] = res.instructions_and_trace[1] if res.instructions_and_trace else None

    recon_pos = np.zeros((N, C, HW), np.float32)
    recon_neg = np.zeros((N, C, HW), np.float32)
    S_pos = np.zeros((N, 162), np.float32)
    Z = np.zeros((N, 162), np.float32)
    for b in range(B):
        r_ = res.results[b]
        oct_idx = octs[b]
        rp = r_["recpos"].reshape(8, 128, KC, 162).transpose(0, 2, 1, 3)
        recon_pos[oct_idx] = rp.reshape(8, C, HW)
        rn = np.asarray(r_["recneg"], dtype=np.float32)
        rn = rn.reshape(8, 128, KC, 162).transpose(0, 2, 1, 3)
        recon_neg[oct_idx] = rn.reshape(8, C, HW)
        S_pos[oct_idx] = r_["srow"][:, 0:162]
        Z[oct_idx] = r_["zrow"]

    # host epilogue: distances, comask, loss
    d_ap = np.sqrt(((f - recon_pos + EPS_DIST) ** 2).sum(axis=1))   # [N, HW]
    d_an = np.sqrt(((f - recon_neg + EPS_DIST) ** 2).sum(axis=1))
    trip = np.maximum(d_ap - d_an + MARGIN, 0.0)

    mask_warp = Z / S_pos
    comask = mask_warp * mask[pos_idx]
    comask = comask - comask.min(axis=1, keepdims=True)
    comask = comask / (comask.max(axis=1, keepdims=True) + EPS_NORM)

    loss = np.float32((comask.sum(axis=0) * trip.sum(axis=0)).sum()
                      / (N * N * HW))
    recon = recon_pos.reshape(N, C, H, W)
    return recon, loss
